# revision 7
# baseline (speedup 1.0000x reference)
"""NetVLAD on 8 Trainium2 NeuronCores — self-contained kernel.

Problem: x [32, 2048, 1024] f32, W [64, 1024] f32, centroids [64, 1024] f32
  -> out [32, 65536] f32  (NetVLAD pooling: per-frame L2 norm, soft-assign
  softmax over 64 clusters, residual aggregation, intra + global L2 norm).

Sharding: data-parallel over batch — 4 samples per core, W/centroids
replicated; no cross-core communication.

v3 design (baseline v1 ~314us, v2 ~402us):
  - x ingested via SWDGE casting DMA f32->bf16 per quarter (one 2MiB-read
    DMA; same SDMA busy time as an f32 HWDGE load but zero engine-side cast
    work). gpsimd engine does nothing else, so the Q7 issue path is free.
  - ONE xbar DMA-transpose per quarter ([128,4096] contiguous source)
    issued on the sync queue: 16 instructions instead of 64, avoiding the
    non-contiguous mid-dim penalty and keeping the ACT queue free.
  - ssq: 2 m-tiles via ACT Square+accum, 2 m-tiles via one DVE
    tensor_mul [128,2048] + one DVE tensor_reduce (engine-balanced).
  - ACT uses only {Ln, Exp, Square, Copy} — all in the
    natural_log_exp_and_others table set, so no ACT_TABLE_LOAD thrash.
  - e = exp(r*z) per tile without accum; sden via one batched DVE 3D
    tensor_reduce per quarter; a = (e*r)/sden via reciprocal + one
    two-scalar tensor_scalar per tile on DVE.
"""

import json

import numpy as np

import concourse.bass as bass
import concourse.mybir as mybir
import concourse.tile as tile

F32 = mybir.dt.float32
BF16 = mybir.dt.bfloat16
AF = mybir.ActivationFunctionType
OP = mybir.AluOpType

B = 32
N_CORES = 8
B_PER_CORE = B // N_CORES
M = 2048
D = 1024
K = 64
NQ = 4           # quarters per sample
TQ = 4           # m-tiles per quarter

_PATCHED = False


def _split_waits_json(bir: dict, max_waits: int = 1) -> dict:
    """Split multi-wait sync infos into standalone EventSemaphore waits.

    The walrus build in this image supports a single sync-wait command per
    instruction, while Tile's sem assignment emits several (e.g. the
    kernel-tail Drain waits on every DMAHW lane). Hoisting the extra waits
    into preceding single-wait EventSemaphore instructions on the same
    engine is semantics-preserving for monotonic semaphores.
    """
    ctr = 0
    for f in bir.get("functions", []):
        for blk in f.get("blocks", []):
            insts = blk.get("instructions", [])
            new = []
            for inst in insts:
                si = inst.get("sync_info")
                waits = si.get("on_wait", []) if si else []
                if len(waits) > max_waits:
                    head, keep = waits[:-max_waits], waits[-max_waits:]
                    for w in head:
                        ctr += 1
                        new.append({
                            "debug": inst.get("debug", 0),
                            "engine": inst["engine"],
                            "ins": [],
                            "name": f"{inst['name']}-wsplit{ctr}",
                            "opcode": "EventSemaphore",
                            "outs": [],
                            "sync_info": {"on_update": [], "on_wait": [w]},
                        })
                    si["on_wait"] = keep
                new.append(inst)
            blk["instructions"] = new
    return bir


def _apply_patch():
    global _PATCHED
    if _PATCHED:
        return
    import concourse.bass_utils as bu
    import concourse.bass2jax as b2j
    orig = bu.compile_bir_kernel

    def patched(bir_json, tmpdir, neff_name="file.neff"):
        d = json.loads(bir_json)
        d = _split_waits_json(d, 1)
        return orig(json.dumps(d).encode(), tmpdir, neff_name)

    bu.compile_bir_kernel = patched
    b2j.compile_bir_kernel = patched

    # The Tile scheduler's cost model gives every DMA an exclusive hold of a
    # single modeled DMA_ENGINES resource, which serializes the x-load queue
    # against the xbar-transpose queue (observed as strict alternation in the
    # HW trace, ~95us of SDMA idle). The real 16 SDMA engines round-robin
    # between queues at packet granularity, and this kernel has no plain
    # SBUF->SBUF DMA (the documented xbar-deadlock pair), so concurrent
    # load/transpose streams are safe. Strip the DMA_ENGINES exclusivity so
    # the scheduler overlaps the queues; correctness is still enforced by the
    # emitted data-dependency semaphores.
    from concourse.cost_model import (
        DeviceAcquire,
        DeviceFree,
        InstructionCostModel,
        NonEngineDevice,
    )
    orig_visit = InstructionCostModel.visit

    def visit(self, instruction, sim):
        timelines = orig_visit(self, instruction, sim)
        out = []
        for tl in timelines:
            out.append([
                ev for ev in tl
                if not (isinstance(ev, (DeviceAcquire, DeviceFree))
                        and getattr(ev, "device", None)
                        == NonEngineDevice.DMA_ENGINES)
            ])
        return out

    InstructionCostModel.visit = visit
    _PATCHED = True


def build_nc():
    nc = bass.Bass()
    x = nc.dram_tensor("x", [B_PER_CORE, M, D], F32, kind="ExternalInput")
    W = nc.dram_tensor("W", [K, D], F32, kind="ExternalInput")
    C = nc.dram_tensor("centroids", [K, D], F32, kind="ExternalInput")
    out = nc.dram_tensor("out", [B_PER_CORE, K * D], F32, kind="ExternalOutput")
    ind2_d = nc.dram_tensor("ind2", [2, 128], F32, kind="ExternalInput")
    indK_d = nc.dram_tensor("indK", [128, 2], F32, kind="ExternalInput")

    xr = x[:, :, :].rearrange("s (q t p) d -> s q p t d", q=NQ, t=TQ, p=128)
    outr = out[:, :].rearrange("s (k d) -> s k d", d=D)

    from contextlib import ExitStack
    with tile.TileContext(nc) as tc, ExitStack() as es:
        singles = es.enter_context(tc.tile_pool(name="singles", bufs=1))
        xbpool = es.enter_context(tc.tile_pool(name="xbp", bufs=7))
        xTpool = es.enter_context(tc.tile_pool(name="xTp", bufs=5))
        sqpool = es.enter_context(tc.tile_pool(name="sqp", bufs=3))
        statpool = es.enter_context(tc.tile_pool(name="statp", bufs=4))
        epool = es.enter_context(tc.tile_pool(name="ep", bufs=2))
        apool = es.enter_context(tc.tile_pool(name="apl", bufs=2))
        rspool = es.enter_context(tc.tile_pool(name="rsp", bufs=4))
        tailpool = es.enter_context(tc.tile_pool(name="tailp", bufs=2))
        zpsum = es.enter_context(tc.tile_pool(name="zps", bufs=2, space="PSUM"))
        aggpsum = es.enter_context(
            tc.tile_pool(name="aggps", bufs=2, space="PSUM"))
        cspsum = es.enter_context(tc.tile_pool(name="csps", bufs=1, space="PSUM"))
        tailpsum = es.enter_context(tc.tile_pool(name="tps", bufs=1, space="PSUM"))

        Wbf = singles.tile([K, D], BF16)
        nc.gpsimd.dma_start(out=Wbf, in_=W[:, :])
        WT = singles.tile([128, 8, K], BF16)  # WT[p, c, k] = W[k, 128c+p]
        nc.sync.dma_start(out=WT, in_=Wbf, transpose=True)
        cpair = singles.tile([128, D], F32)
        nc.gpsimd.dma_start(out=cpair[0:64, :], in_=C[:, :])
        nc.gpsimd.dma_start(out=cpair[64:128, :], in_=C[:, :])
        ind2 = singles.tile([2, 128], F32)
        nc.sync.dma_start(out=ind2, in_=ind2_d[:, :])
        indK = singles.tile([128, 2], F32)
        nc.sync.dma_start(out=indK, in_=indK_d[:, :])

        def quarter(s, q, agg, cs):
            base = 64 * (s % 2)
            first = q == 0
            last = q == NQ - 1

            # 1) casting quarter load (SWDGE): f32 DRAM -> bf16 SBUF
            xb = xbpool.tile([128, TQ, D], BF16, tag="xb", name=f"xb_{s}_{q}")
            nc.gpsimd.dma_start(out=xb, in_=xr[s, q])
            xb_flat = xb[:, :, :].rearrange("p t d -> p (t d)")

            # 2) per-quarter xbar transpose (one instruction, sync queue)
            #    xT[p, t, c, m] = xb[m, t, 128c + p]
            xT = xTpool.tile([128, TQ, 8, 128], BF16, tag="xT",
                             name=f"xT_{s}_{q}")
            xT3 = xT[:, :, :, :].rearrange("p t c m -> p (t c) m")
            nc.sync.dma_start(out=xT3, in_=xb_flat, transpose=True)

            # 3) ssq: tiles 0-1 on ACT (Square+accum), tiles 2-3 on DVE
            #    (one tensor_mul [128,2048] + one tensor_reduce)
            ssq = statpool.tile([128, TQ], F32, tag="ssq", name=f"ssq_{s}_{q}")
            for i in range(2):
                sqa = sqpool.tile([128, D], BF16, tag="sqa",
                                  name=f"sqa_{s}_{q}_{i}")
                nc.scalar.activation(
                    out=sqa, in_=xb[:, i, :], func=AF.Square,
                    accum_out=ssq[:, i:i + 1],
                )
            sqd = sqpool.tile([128, 2, D], BF16, tag="sqd",
                              name=f"sqd_{s}_{q}")
            sqd_flat = sqd[:, :, :].rearrange("p t d -> p (t d)")
            xb23 = xb[:, 2:4, :].rearrange("p t d -> p (t d)")
            nc.vector.tensor_mul(sqd_flat, xb23, xb23)
            nc.vector.reduce_sum(
                out=ssq[:, 2:4], in_=sqd, axis=mybir.AxisListType.X)

            # 4) stats (ACT, all in natural_log_exp table set):
            #    lnt = ln(ssq); invr = ||x|| = exp(0.5 lnt) (bf16 for PE);
            #    r = 1/||x|| = exp(-0.5 lnt)
            lnt = statpool.tile([128, TQ], F32, tag="lnt", name=f"lnt_{s}_{q}")
            nc.scalar.activation(out=lnt, in_=ssq, func=AF.Ln)
            invr = statpool.tile([128, TQ], BF16, tag="invr",
                                 name=f"invr_{s}_{q}")
            nc.scalar.activation(out=invr, in_=lnt, func=AF.Exp, scale=0.5)
            r = statpool.tile([128, TQ], F32, tag="r", name=f"r_{s}_{q}")
            nc.scalar.activation(out=r, in_=lnt, func=AF.Exp, scale=-0.5)

            # 5) logits z[m, k] via PE, contract d in 8 chunks
            zq = zpsum.tile([128, TQ, K], F32, tag="zq", name=f"zq_{s}_{q}")
            for i in range(TQ):
                for c in range(8):
                    nc.tensor.matmul(
                        zq[:, i, :], lhsT=xT[:, i, c, :], rhs=WT[:, c, :],
                        start=(c == 0), stop=(c == 7),
                    )

            # 6) e = exp(z * r) per tile (ACT), then one batched sden reduce
            e = epool.tile([128, TQ, K], BF16, tag="e", name=f"e_{s}_{q}")
            for i in range(TQ):
                nc.scalar.activation(
                    out=e[:, i, :], in_=zq[:, i, :], func=AF.Exp,
                    scale=r[:, i:i + 1],
                )
            sden = statpool.tile([128, TQ], F32, tag="sden",
                                 name=f"sden_{s}_{q}")
            nc.vector.reduce_sum(
                out=sden, in_=e, axis=mybir.AxisListType.X)

            # 7) a = e * r / sden  [softmax(r z) * (1/||x||)]
            rsd = statpool.tile([128, TQ], F32, tag="rsd", name=f"rsd_{s}_{q}")
            nc.vector.reciprocal(out=rsd, in_=sden)
            a = apool.tile([128, TQ, K], BF16, tag="a", name=f"a_{s}_{q}")
            for i in range(TQ):
                nc.vector.tensor_scalar(
                    out=a[:, i, :], in0=e[:, i, :],
                    scalar1=r[:, i:i + 1], scalar2=rsd[:, i:i + 1],
                    op0=OP.mult, op1=OP.mult,
                )

            # 8) agg += a^T x ; cs += a^T ||x||
            for i in range(TQ):
                st_ = first and i == 0
                sp_ = last and i == TQ - 1
                nc.tensor.matmul(
                    agg[base:base + 64, 0:512], lhsT=a[:, i, :],
                    rhs=xb[:, i, 0:512], start=st_, stop=sp_,
                )
                nc.tensor.matmul(
                    agg[base:base + 64, 512:1024], lhsT=a[:, i, :],
                    rhs=xb[:, i, 512:1024], start=st_, stop=sp_,
                )
                nc.tensor.matmul(
                    cs[base:base + 64, 0:1], lhsT=a[:, i, :],
                    rhs=invr[:, i:i + 1], start=st_, stop=sp_,
                )

        def tail_pair(p, agg, cs):
            sa, sb = 2 * p, 2 * p + 1
            cssb = rspool.tile([128, 1], F32, tag="cssb", name=f"cssb_{p}")
            nc.vector.tensor_copy(out=cssb, in_=cs[:, 0:1])
            tmp = tailpool.tile([128, D], F32, tag="tmp", name=f"tmp_{p}")
            nc.vector.tensor_scalar_mul(tmp, cpair, cssb)
            vlad = tailpool.tile([128, D], F32, tag="vlad", name=f"vlad_{p}")
            nc.vector.tensor_sub(vlad, agg[:, :], tmp)
            sq2 = sqpool.tile([128, D], BF16, tag="sqa", name=f"sqt_{p}")
            vssq = rspool.tile([128, 1], F32, tag="vssq", name=f"vssq_{p}")
            nc.scalar.activation(out=sq2, in_=vlad, func=AF.Square,
                                 accum_out=vssq)
            lnv = rspool.tile([128, 1], F32, tag="lnv", name=f"lnv_{p}")
            nc.scalar.activation(out=lnv, in_=vssq, func=AF.Ln)
            rv = rspool.tile([128, 1], F32, tag="rv", name=f"rv_{p}")
            nc.scalar.activation(out=rv, in_=lnv, func=AF.Exp, scale=-0.5)
            ssqn = rspool.tile([128, 1], F32, tag="ssqn", name=f"ssqn_{p}")
            nc.vector.tensor_scalar(
                out=ssqn, in0=vssq, scalar1=rv, scalar2=rv,
                op0=OP.mult, op1=OP.mult,
            )
            gsum = tailpsum.tile([2, 2], F32, tag="tps", name=f"gsum_{p}")
            nc.tensor.matmul(gsum[:, 0:1], lhsT=indK, rhs=ssqn,
                             start=True, stop=True)
            lng = rspool.tile([2, 1], F32, tag="lng", name=f"lng_{p}")
            nc.scalar.activation(out=lng, in_=gsum[:, 0:1], func=AF.Ln)
            ginv = rspool.tile([2, 1], F32, tag="ginv", name=f"ginv_{p}")
            nc.scalar.activation(out=ginv, in_=lng, func=AF.Exp, scale=-0.5)
            gb = tailpsum.tile([128, 2], F32, tag="tps", name=f"gb_{p}")
            nc.tensor.matmul(gb[:, 0:1], lhsT=ind2, rhs=ginv,
                             start=True, stop=True)
            fs = rspool.tile([128, 1], F32, tag="fs", name=f"fs_{p}")
            nc.vector.tensor_mul(fs, rv, gb[:, 0:1])
            osb = tailpool.tile([128, D], F32, tag="osb", name=f"osb_{p}")
            nc.vector.tensor_scalar_mul(osb, vlad, fs)
            nc.scalar.dma_start(out=outr[sa], in_=osb[0:64, :])
            nc.scalar.dma_start(out=outr[sb], in_=osb[64:128, :])

        for p in range(2):
            agg = aggpsum.tile([128, D], F32, tag="agg", name=f"agg_{p}")
            cs = cspsum.tile([128, 8], F32, tag="cs", name=f"cs_{p}")
            for s in (2 * p, 2 * p + 1):
                for q in range(NQ):
                    quarter(s, q, agg, cs)
            tail_pair(p, agg, cs)

    return nc


_NC_CACHE = None


def kernel(**inputs: np.ndarray) -> np.ndarray:
    global _NC_CACHE
    _apply_patch()
    from concourse.bass_utils import run_bass_kernel_spmd

    x = np.ascontiguousarray(np.asarray(inputs["x"], dtype=np.float32))
    W = np.ascontiguousarray(np.asarray(inputs["W"], dtype=np.float32))
    cent = np.ascontiguousarray(
        np.asarray(inputs["centroids"], dtype=np.float32))

    ind2 = np.zeros((2, 128), dtype=np.float32)
    ind2[0, 0:64] = 1.0
    ind2[1, 64:128] = 1.0
    indK = np.zeros((128, 2), dtype=np.float32)
    indK[0:64, 0] = 1.0
    indK[64:128, 1] = 1.0

    if _NC_CACHE is None:
        _NC_CACHE = build_nc()
    nc = _NC_CACHE

    in_maps = [
        {
            "x": np.ascontiguousarray(
                x[B_PER_CORE * c:B_PER_CORE * (c + 1)]),
            "W": W,
            "centroids": cent,
            "ind2": ind2,
            "indK": indK,
        }
        for c in range(N_CORES)
    ]
    res = run_bass_kernel_spmd(nc, in_maps, core_ids=list(range(N_CORES)))
    return np.concatenate([r["out"] for r in res.results], axis=0)
